# revision 41
# baseline (speedup 1.0000x reference)
"""Trainium2 Bass kernel: causal multi-head self-attention (B=4, T=4096, D=128, H=4, dh=32).

Sharding: 8 cores = 4 batches x 2 head-pairs. Core c handles batch c//2, heads
{2*(c%2), 2*(c%2)+1}. Each core emits per-head unnormalized projections Y_h and
softmax denominators l_h; the host computes sum_h Y_h / l_h per batch.

All matmuls bf16/fp16, untiled full-array (keeps the PE HAM clock-gate warm at
2.4 GHz — row/col-tiled matmuls don't register as PE activity and the clock
throttles to 1.2 GHz). Per (head, q-super of 512 queries), j-blocks descending:
  S^T[j,q]   = ktz_jb(zero-padded K=128) @ qt  -> PSUM (diag blocks sliced to
               alive columns)
  E          : head 0 of the pair on ScalarE (exp); head 1 on VectorE via a
               custom DVE op relu(s*a+b) -> int16 whose bits read as bf16 give
               2^(s*log2e) (Schraudolph).
  causal mask: gpsimd affine_select zeroes j>q entries of E on diagonal blocks
               (emitted right after E, overlapped with later groups).
  O^T       += [V_j | ones | 0] @ E  (row 32 = softmax denominator l)
  proj       : Y[q,:128], l[q] = osb_block^T @ [W_out^T | ones-at-row-32]
"""

import math
import numpy as np

import concourse.bass as bass
import concourse.bacc as bacc
import concourse.mybir as mybir
import concourse.tile as tile
from concourse import bass_utils
import concourse.dve_ops as dve_ops
from concourse.dve_spec import Spec, Src0, C0, C1, relu, lower
from concourse.dve_uop import DveOpSpec

F32 = mybir.dt.float32
BF16 = mybir.dt.bfloat16
FP16 = mybir.dt.float16
I16 = mybir.dt.int16
Exp = mybir.ActivationFunctionType.Exp

B, T, D = 4, 4096, 128
H, DH = 4, 32
NCORES = 8
NQS = T // 512
SCALE = 1.0 / math.sqrt(DH)

# Schraudolph bf16-bit exp: bf16_bits(e^s) ~= round(s*log2e*128 + (127+sigma)*128)
SIGMA = -0.03
SCHR_A = (1.0 / math.log(2.0)) * SCALE * 128.0
SCHR_B = (127.0 + SIGMA) * 128.0


def _register_exp2():
    name = "EXP2_BITS_ANT"
    for op in dve_ops.OPS:
        if op.name == name:
            return op
    spec = Spec(body=relu(Src0 * C0 + C1))
    row = dve_ops._CUSTOM_DVE_ROW_BASE + len(dve_ops.OPS)
    assert row < 0x20
    shas = {}
    for ver in ("v3", "v4"):
        try:
            s = DveOpSpec(name=name, opcode=row, uops=lower(spec, ver=ver), rd1_en=False)
            shas[ver] = s.sha(ver)
        except Exception:
            pass
    dve_ops._SUB_OPCODE_FOR_NAME[name] = row
    op = dve_ops.DveOp(name, spec, subdim=False, uops_sha=shas)
    dve_ops.OPS.append(op)
    dve_ops.CUSTOM_DVE_SPECS[name] = spec
    return op


EXP2_OP = _register_exp2()


def build_program() -> bacc.Bacc:
    nc = bacc.Bacc("TRN2", target_bir_lowering=False, debug=False, num_devices=NCORES)

    xt_d = nc.dram_tensor("xt", [D, T], BF16, kind="ExternalInput").ap()
    wqk_d = [nc.dram_tensor(f"wqk{h}", [D, 64], BF16, kind="ExternalInput").ap() for h in range(2)]
    wv_d = nc.dram_tensor("wv", [D, 64], BF16, kind="ExternalInput").ap()
    wo_d = [nc.dram_tensor(f"wo{h}", [128, 132], FP16, kind="ExternalInput").ap() for h in range(2)]
    zpad_d = nc.dram_tensor("zpad", [96, T], BF16, kind="ExternalInput").ap()
    y_d = nc.dram_tensor("y", [2, NQS * 4, 128, 132], FP16, kind="ExternalOutput").ap()

    with tile.TileContext(nc) as tc:
        with (
            tc.tile_pool(name="const", bufs=1) as cpool,
            tc.tile_pool(name="epool", bufs=6) as epool,
            tc.tile_pool(name="ypool", bufs=4) as ypool,
            tc.tile_pool(name="psS", bufs=3, space="PSUM") as psS,
            tc.tile_pool(name="psO", bufs=2, space="PSUM") as psO,
        ):
            # ---- persistent SBUF ----
            xt = cpool.tile([D, T], BF16)
            wqk = [cpool.tile([D, 64], BF16, name=f"wqk{h}", tag=f"wqk{h}") for h in range(2)]
            wv = cpool.tile([D, 64], BF16)
            wo = [cpool.tile([128, 132], FP16, name=f"wo{h}", tag=f"wo{h}") for h in range(2)]
            qt = [cpool.tile([128, T], BF16, name=f"qt{h}", tag=f"qt{h}") for h in range(2)]
            ktz = [cpool.tile([128, T], BF16, name=f"ktz{h}", tag=f"ktz{h}") for h in range(2)]
            vx = [cpool.tile([128, 128 * 32], BF16, name=f"vx{h}", tag=f"vx{h}") for h in range(2)]
            osb = [cpool.tile([128, T], FP16, name=f"osb{h}", tag=f"osb{h}") for h in range(2)]

            # ---- init loads ----
            nc.sync.dma_start(xt[:, 0:512], xt_d[:, 0:512])
            for h in range(2):
                nc.sync.dma_start(wqk[h][:, :], wqk_d[h][:, :])
            for h in range(2):
                nc.scalar.dma_start(wo[h][:, :], wo_d[h][:, :])
            nc.scalar.dma_start(wv[:, :], wv_d[:, :])
            zq = [nc.sync, nc.gpsimd, nc.sync, nc.gpsimd]
            for h in range(2):
                # zero the padded contraction rows once; Q/K copies only write
                # rows 0:32. Issued from four different engine queues so the
                # descriptors dispatch in parallel instead of serializing the
                # startup on the Sync queue.
                zq[2 * h].dma_start(qt[h][32:128, :], zpad_d[:, :])
                zq[2 * h + 1].dma_start(ktz[h][32:128, :], zpad_d[:, :])
                # vx pattern: [V_j | ones | zeros] per 128-col block.
                # Unit 0's region (blocks 0-3) first so O(qs=0) isn't blocked;
                # the bulk zeroing goes to VectorE which is idle at startup.
                nc.gpsimd.memset(vx[h][:, 0:512], 0.0)
                for jb in range(4):
                    nc.gpsimd.memset(vx[h][:, 128 * jb + 32 : 128 * jb + 33], 1.0)
                nc.vector.memset(vx[h][:, 512:4096], 0.0)
                for jb in range(4, 32):
                    nc.gpsimd.memset(vx[h][:, 128 * jb + 32 : 128 * jb + 33], 1.0)

            def copy_h(h, out, in_):
                """PSUM->SBUF evacuations: head 0 on ScalarE, head 1 on VectorE."""
                if h == 0:
                    nc.scalar.copy(out, in_)
                else:
                    nc.vector.tensor_copy(out, in_)

            def emit_qkv(qs):
                qsl = slice(512 * qs, 512 * (qs + 1))
                for h in range(2):
                    p = psS.tile([128, 1024], F32, name="p", tag="s")
                    nc.tensor.matmul(p[0:64, 0:512], wqk[h][:, :], xt[:, qsl], start=True, stop=True)
                    # Q/K evacuations split by head: ScalarE is the period
                    # setter now (exp + aux), so h1's copies go to VectorE
                    copy_h(h, qt[h][0:32, qsl], p[0:32, 0:512])
                    copy_h(h, ktz[h][0:32, qsl], p[32:64, 0:512])
                pv = psS.tile([128, 1024], F32, name="pv", tag="s")
                for k in range(4):
                    jsl = slice(512 * qs + 128 * k, 512 * qs + 128 * (k + 1))
                    nc.tensor.matmul(pv[:, 64 * k : 64 * k + 64], xt[:, jsl], wv[:, :], start=True, stop=True)
                for h in range(2):
                    src = pv[:, 0:256].rearrange("p (n s) -> p n s", s=64)[:, :, 32 * h : 32 * h + 32]
                    dst = vx[h][:, 512 * qs * 1 : 512 * (qs + 1)].rearrange("p (n s) -> p n s", s=128)[:, :, 0:32]
                    nc.vector.tensor_copy(dst, src)
                if qs + 1 < NQS:
                    nsl = slice(512 * (qs + 1), 512 * (qs + 2))
                    nc.sync.dma_start(xt[:, nsl], xt_d[:, nsl])

            def emit_attn(qs, mid_cb=None):
                njb = 4 * (qs + 1)
                npairs = njb // 2
                o_ps = [psO.tile([128, 512], F32, name=f"o{h}", tag="o") for h in range(2)]
                s_tiles = {}
                e_tiles = {}
                blocks = list(range(njb - 1, -1, -1))  # descending: diag first
                pairs = [blocks[2 * i : 2 * i + 2] for i in range(npairs)]

                def emit_S(h, gi):
                    s = psS.tile([128, 1024], F32, name="s", tag="s")
                    s_tiles[(h, gi)] = s
                    for k, jb in enumerate(pairs[gi]):
                        g = jb - 4 * qs
                        lo = 128 * g if g > 0 else 0
                        nc.tensor.matmul(
                            s[:, 512 * k + lo : 512 * (k + 1)],
                            ktz[h][:, 128 * jb : 128 * (jb + 1)],
                            qt[h][:, 512 * qs + lo : 512 * (qs + 1)],
                            start=True,
                            stop=True,
                        )

                def emit_E(h, gi):
                    s = s_tiles.pop((h, gi))
                    e = epool.tile([128, 1024], BF16, name="e", tag="e")
                    e_tiles[(h, gi)] = e
                    if h == 0:
                        nc.scalar.activation(e[:, :], s[:, :], Exp, scale=SCALE)
                    else:
                        nc.vector._custom_dve(
                            EXP2_OP, out=e[:, :].bitcast(I16), in0=s[:, :], s0=SCHR_A, s1=SCHR_B
                        )
                    # causal mask: zero E where 128g + p > l on diagonal blocks
                    for k, jb in enumerate(pairs[gi]):
                        g = jb - 4 * qs
                        if g >= 0:
                            w = 128 * (g + 1)
                            nc.gpsimd.affine_select(
                                e[:, 512 * k : 512 * k + w],
                                e[:, 512 * k : 512 * k + w],
                                pattern=[[1, w]],
                                compare_op=mybir.AluOpType.is_ge,
                                fill=0.0,
                                base=-128 * g,
                                channel_multiplier=-1,
                            )

                def emit_O(h, gi):
                    e = e_tiles.pop((h, gi))
                    # 2x col-tiled: block k accumulates into partition strip
                    # [64k, 64k+64); strips are summed for free in the output
                    # projection via W_out replicated along partitions.
                    for k, jb in enumerate(pairs[gi]):
                        nc.tensor.matmul(
                            o_ps[h][64 * k : 64 * (k + 1), :],
                            vx[h][:, 128 * jb : 128 * jb + 64],
                            e[:, 512 * k : 512 * (k + 1)],
                            start=(gi == 0),
                            stop=(gi == npairs - 1),
                        )

                emit_S(0, 0)
                emit_S(1, 0)
                for gi in range(npairs):
                    emit_E(0, gi)
                    emit_E(1, gi)
                    if gi + 1 < npairs:
                        emit_S(0, gi + 1)
                        emit_S(1, gi + 1)
                    emit_O(0, gi)
                    # O(h1) deferred one period: by then its Schraudolph E is
                    # long done, so the in-order PE queue never stalls on VectorE
                    if gi > 0:
                        emit_O(1, gi - 1)
                    if mid_cb is not None and gi == npairs // 2:
                        mid_cb()
                emit_O(1, npairs - 1)
                return o_ps

            def emit_osb(qs, o_ps):
                qsl = slice(512 * qs, 512 * (qs + 1))
                for h in range(2):
                    # both osb evacuations on VectorE: on ScalarE this copy sits
                    # just ahead of the next unit's first exp and blocks it
                    nc.vector.tensor_copy(osb[h][:, qsl], o_ps[h][:, :])

            def emit_proj(qs):
                for h in range(2):
                    p = psS.tile([128, 1024], F32, name="pp", tag="s")
                    for lqb in range(4):
                        qb = 4 * qs + lqb
                        nc.tensor.matmul(
                            p[:, 256 * lqb : 256 * lqb + 132],
                            osb[h][:, 128 * qb : 128 * (qb + 1)],
                            wo[h][:, :],
                            start=True,
                            stop=True,
                        )
                    yb = ypool.tile([128, 4, 132], FP16, name="yb", tag="y")
                    src = p[:, 0:1024].rearrange("p (n s) -> p n s", s=256)[:, :, 0:132]
                    # y evacuations on ScalarE: it idles at unit boundaries
                    # while VectorE (schr + most copies) is the saturated engine
                    nc.scalar.copy(yb[:, :, :], src)
                    dst = y_d[h, 4 * qs : 4 * qs + 4].rearrange("n p c -> p n c")
                    nc.sync.dma_start(dst, yb[:, :, :])

            with nc.named_scope("attn"):
                o_prev = None
                for qs in range(NQS):
                    if qs == 0:
                        emit_qkv(0)
                    if qs > 0:
                        emit_osb(qs - 1, o_prev)
                    # next q-super's qkv is emitted mid-unit so its PSUM->SBUF
                    # copies overlap this unit instead of stalling the boundary
                    cb = (lambda q=qs: emit_qkv(q + 1)) if qs + 1 < NQS else None
                    o_cur = emit_attn(qs, cb)
                    if qs > 0:
                        emit_proj(qs - 1)
                    o_prev = o_cur
                emit_osb(NQS - 1, o_prev)
                emit_proj(NQS - 1)

    nc.compile()
    return nc


def _to_bf16(x: np.ndarray) -> np.ndarray:
    import ml_dtypes

    return np.ascontiguousarray(x, dtype=np.float32).astype(ml_dtypes.bfloat16)


def make_in_maps(x: np.ndarray, W_qkv: np.ndarray, W_out: np.ndarray):
    x = np.asarray(x, dtype=np.float32)
    W_qkv = np.asarray(W_qkv, dtype=np.float32)
    W_out = np.asarray(W_out, dtype=np.float32)

    in_maps = []
    for c in range(NCORES):
        b = c // 2
        h0 = 2 * (c % 2)
        m = {"xt": _to_bf16(x[b].T), "zpad": _to_bf16(np.zeros((96, T), np.float32))}
        for i in range(2):
            h = h0 + i
            wqk = np.zeros((D, 64), np.float32)
            wqk[:, 0:32] = W_qkv[32 * h : 32 * h + 32, :].T
            wqk[:, 32:64] = W_qkv[128 + 32 * h : 128 + 32 * h + 32, :].T
            m[f"wqk{i}"] = _to_bf16(wqk)
            woi = np.zeros((128, 132), np.float32)
            woi[0:32, 0:128] = W_out[:, 32 * h : 32 * h + 32].T
            woi[64:96, 0:128] = W_out[:, 32 * h : 32 * h + 32].T
            woi[32, 128] = 1.0
            woi[96, 128] = 1.0
            m[f"wo{i}"] = woi.astype(np.float16)
        m["wv"] = _to_bf16(W_qkv[256 + 32 * h0 : 256 + 32 * h0 + 64, :].T)
        in_maps.append(m)
    return in_maps


_PROGRAM_CACHE = {}


def kernel(x: np.ndarray, W_qkv: np.ndarray, W_out: np.ndarray, _trace=False, _tmpdir=None) -> np.ndarray:
    if "nc" not in _PROGRAM_CACHE:
        _PROGRAM_CACHE["nc"] = build_program()
    nc = _PROGRAM_CACHE["nc"]

    in_maps = make_in_maps(x, W_qkv, W_out)
    res = bass_utils.run_bass_kernel_spmd(
        nc, in_maps, core_ids=list(range(NCORES)), trace=_trace, tmpdir=_tmpdir
    )
    out = np.zeros((B, T, D), np.float32)
    for c in range(NCORES):
        b = c // 2
        y = np.asarray(res.results[c]["y"], dtype=np.float32)  # [2, 32, 128, 132]
        for i in range(2):
            yi = y[i].reshape(T, 132)
            out[b] += yi[:, 0:128] / yi[:, 128:129]
    if _trace:
        kernel.last_result = res
    return out


# revision 43
# speedup vs baseline: 1.0247x; 1.0247x over previous
"""Trainium2 Bass kernel: causal multi-head self-attention (B=4, T=4096, D=128, H=4, dh=32).

Sharding: 8 cores = 4 batches x 2 head-pairs. Core c handles batch c//2, heads
{2*(c%2), 2*(c%2)+1}. Each core emits per-head unnormalized projections Y_h and
softmax denominators l_h; the host computes sum_h Y_h / l_h per batch.

All matmuls bf16/fp16, untiled full-array (keeps the PE HAM clock-gate warm at
2.4 GHz — row/col-tiled matmuls don't register as PE activity and the clock
throttles to 1.2 GHz). Per (head, q-super of 512 queries), j-blocks descending:
  S^T[j,q]   = ktz_jb(zero-padded K=128) @ qt  -> PSUM (diag blocks sliced to
               alive columns)
  E          : head 0 of the pair on ScalarE (exp); head 1 on VectorE via a
               custom DVE op relu(s*a+b) -> int16 whose bits read as bf16 give
               2^(s*log2e) (Schraudolph).
  causal mask: gpsimd affine_select zeroes j>q entries of E on diagonal blocks
               (emitted right after E, overlapped with later groups).
  O^T       += [V_j | ones | 0] @ E  (row 32 = softmax denominator l)
  proj       : Y[q,:128], l[q] = osb_block^T @ [W_out^T | ones-at-row-32]
"""

import math
import numpy as np

import concourse.bass as bass
import concourse.bacc as bacc
import concourse.mybir as mybir
import concourse.tile as tile
from concourse import bass_utils
import concourse.dve_ops as dve_ops
from concourse.dve_spec import Spec, Src0, C0, C1, relu, lower
from concourse.dve_uop import DveOpSpec

F32 = mybir.dt.float32
BF16 = mybir.dt.bfloat16
FP16 = mybir.dt.float16
I16 = mybir.dt.int16
Exp = mybir.ActivationFunctionType.Exp

B, T, D = 4, 4096, 128
H, DH = 4, 32
NCORES = 8
NQS = T // 512
SCALE = 1.0 / math.sqrt(DH)

# Schraudolph bf16-bit exp: bf16_bits(e^s) ~= round(s*log2e*128 + (127+sigma)*128)
SIGMA = -0.03
SCHR_A = (1.0 / math.log(2.0)) * SCALE * 128.0
SCHR_B = (127.0 + SIGMA) * 128.0


def _register_exp2():
    name = "EXP2_BITS_ANT"
    for op in dve_ops.OPS:
        if op.name == name:
            return op
    spec = Spec(body=relu(Src0 * C0 + C1))
    row = dve_ops._CUSTOM_DVE_ROW_BASE + len(dve_ops.OPS)
    assert row < 0x20
    shas = {}
    for ver in ("v3", "v4"):
        try:
            s = DveOpSpec(name=name, opcode=row, uops=lower(spec, ver=ver), rd1_en=False)
            shas[ver] = s.sha(ver)
        except Exception:
            pass
    dve_ops._SUB_OPCODE_FOR_NAME[name] = row
    op = dve_ops.DveOp(name, spec, subdim=False, uops_sha=shas)
    dve_ops.OPS.append(op)
    dve_ops.CUSTOM_DVE_SPECS[name] = spec
    return op


EXP2_OP = _register_exp2()


def build_program() -> bacc.Bacc:
    nc = bacc.Bacc("TRN2", target_bir_lowering=False, debug=False, num_devices=NCORES)

    xt_d = nc.dram_tensor("xt", [D, T], BF16, kind="ExternalInput").ap()
    wqk_d = [nc.dram_tensor(f"wqk{h}", [D, 64], BF16, kind="ExternalInput").ap() for h in range(2)]
    wv_d = nc.dram_tensor("wv", [D, 64], BF16, kind="ExternalInput").ap()
    wo_d = [nc.dram_tensor(f"wo{h}", [128, 132], FP16, kind="ExternalInput").ap() for h in range(2)]
    zpad_d = nc.dram_tensor("zpad", [96, T], BF16, kind="ExternalInput").ap()
    y_d = nc.dram_tensor("y", [2, NQS * 4, 128, 132], FP16, kind="ExternalOutput").ap()

    with tile.TileContext(nc) as tc:
        with (
            tc.tile_pool(name="const", bufs=1) as cpool,
            tc.tile_pool(name="epool", bufs=6) as epool,
            tc.tile_pool(name="ypool", bufs=4) as ypool,
            tc.tile_pool(name="psS", bufs=3, space="PSUM") as psS,
            tc.tile_pool(name="psO", bufs=2, space="PSUM") as psO,
        ):
            # ---- persistent SBUF ----
            xt = cpool.tile([D, T], BF16)
            wqk = [cpool.tile([D, 64], BF16, name=f"wqk{h}", tag=f"wqk{h}") for h in range(2)]
            wv = cpool.tile([D, 64], BF16)
            wo = [cpool.tile([128, 132], FP16, name=f"wo{h}", tag=f"wo{h}") for h in range(2)]
            qt = [cpool.tile([128, T], BF16, name=f"qt{h}", tag=f"qt{h}") for h in range(2)]
            ktz = [cpool.tile([128, T], BF16, name=f"ktz{h}", tag=f"ktz{h}") for h in range(2)]
            vx = [cpool.tile([128, 128 * 32], BF16, name=f"vx{h}", tag=f"vx{h}") for h in range(2)]
            osb = [cpool.tile([128, T], FP16, name=f"osb{h}", tag=f"osb{h}") for h in range(2)]

            # ---- init loads ----
            nc.sync.dma_start(xt[:, 0:512], xt_d[:, 0:512])
            for h in range(2):
                nc.sync.dma_start(wqk[h][:, :], wqk_d[h][:, :])
            for h in range(2):
                nc.scalar.dma_start(wo[h][:, :], wo_d[h][:, :])
            nc.scalar.dma_start(wv[:, :], wv_d[:, :])
            zq = [nc.sync, nc.gpsimd, nc.sync, nc.gpsimd]
            for h in range(2):
                # zero the padded contraction rows once; Q/K copies only write
                # rows 0:32. Issued from four different engine queues so the
                # descriptors dispatch in parallel instead of serializing the
                # startup on the Sync queue.
                zq[2 * h].dma_start(qt[h][32:128, :], zpad_d[:, :])
                zq[2 * h + 1].dma_start(ktz[h][32:128, :], zpad_d[:, :])
                # vx pattern: [V_j | ones | zeros] per 128-col block.
                # Unit 0's region (blocks 0-3) first so O(qs=0) isn't blocked;
                # the bulk zeroing goes to VectorE which is idle at startup.
                nc.gpsimd.memset(vx[h][:, 0:512], 0.0)
                for jb in range(4):
                    nc.gpsimd.memset(vx[h][:, 128 * jb + 32 : 128 * jb + 33], 1.0)
                nc.vector.memset(vx[h][:, 512:4096], 0.0)
                for jb in range(4, 32):
                    nc.gpsimd.memset(vx[h][:, 128 * jb + 32 : 128 * jb + 33], 1.0)

            def copy_h(h, out, in_):
                """PSUM->SBUF evacuations: head 0 on ScalarE, head 1 on VectorE."""
                if h == 0:
                    nc.scalar.copy(out, in_)
                else:
                    nc.vector.tensor_copy(out, in_)

            def emit_qkv(qs):
                qsl = slice(512 * qs, 512 * (qs + 1))
                for h in range(2):
                    p = psS.tile([128, 1024], F32, name="p", tag="s")
                    nc.tensor.matmul(p[0:64, 0:512], wqk[h][:, :], xt[:, qsl], start=True, stop=True)
                    # Q/K evacuations split by head: ScalarE is the period
                    # setter now (exp + aux), so h1's copies go to VectorE
                    copy_h(h, qt[h][0:32, qsl], p[0:32, 0:512])
                    copy_h(h, ktz[h][0:32, qsl], p[32:64, 0:512])
                pv = psS.tile([128, 1024], F32, name="pv", tag="s")
                for k in range(4):
                    jsl = slice(512 * qs + 128 * k, 512 * qs + 128 * (k + 1))
                    nc.tensor.matmul(pv[:, 64 * k : 64 * k + 64], xt[:, jsl], wv[:, :], start=True, stop=True)
                for h in range(2):
                    src = pv[:, 0:256].rearrange("p (n s) -> p n s", s=64)[:, :, 32 * h : 32 * h + 32]
                    dst = vx[h][:, 512 * qs * 1 : 512 * (qs + 1)].rearrange("p (n s) -> p n s", s=128)[:, :, 0:32]
                    nc.vector.tensor_copy(dst, src)
                if qs + 1 < NQS:
                    nsl = slice(512 * (qs + 1), 512 * (qs + 2))
                    nc.sync.dma_start(xt[:, nsl], xt_d[:, nsl])

            def emit_attn(qs, mid_cb=None):
                njb = 4 * (qs + 1)
                npairs = njb // 2
                o_ps = [psO.tile([128, 512], F32, name=f"o{h}", tag="o") for h in range(2)]
                s_tiles = {}
                e_tiles = {}
                blocks = list(range(njb - 1, -1, -1))  # descending: diag first
                pairs = [blocks[2 * i : 2 * i + 2] for i in range(npairs)]

                def emit_S(h, gi):
                    s = psS.tile([128, 1024], F32, name="s", tag="s")
                    s_tiles[(h, gi)] = s
                    for k, jb in enumerate(pairs[gi]):
                        g = jb - 4 * qs
                        lo = 128 * g if g > 0 else 0
                        nc.tensor.matmul(
                            s[:, 512 * k + lo : 512 * (k + 1)],
                            ktz[h][:, 128 * jb : 128 * (jb + 1)],
                            qt[h][:, 512 * qs + lo : 512 * (qs + 1)],
                            start=True,
                            stop=True,
                        )

                def emit_E(h, gi):
                    s = s_tiles.pop((h, gi))
                    e = epool.tile([128, 1024], BF16, name="e", tag="e")
                    e_tiles[(h, gi)] = e
                    if h == 0:
                        nc.scalar.activation(e[:, :], s[:, :], Exp, scale=SCALE)
                    elif gi == 0:
                        # first pair = diag blocks g=3,2: columns below the
                        # causal frontier are overwritten to 0 by affine_select
                        # anyway, so the Schraudolph op skips them (384 of 1024)
                        for sl in (slice(384, 512), slice(768, 1024)):
                            nc.vector._custom_dve(
                                EXP2_OP, out=e[:, sl].bitcast(I16), in0=s[:, sl], s0=SCHR_A, s1=SCHR_B
                            )
                    else:
                        nc.vector._custom_dve(
                            EXP2_OP, out=e[:, :].bitcast(I16), in0=s[:, :], s0=SCHR_A, s1=SCHR_B
                        )
                    # causal mask: zero E where 128g + p > l on diagonal blocks
                    for k, jb in enumerate(pairs[gi]):
                        g = jb - 4 * qs
                        if g >= 0:
                            w = 128 * (g + 1)
                            nc.gpsimd.affine_select(
                                e[:, 512 * k : 512 * k + w],
                                e[:, 512 * k : 512 * k + w],
                                pattern=[[1, w]],
                                compare_op=mybir.AluOpType.is_ge,
                                fill=0.0,
                                base=-128 * g,
                                channel_multiplier=-1,
                            )

                def emit_O(h, gi):
                    e = e_tiles.pop((h, gi))
                    # 2x col-tiled: block k accumulates into partition strip
                    # [64k, 64k+64); strips are summed for free in the output
                    # projection via W_out replicated along partitions.
                    for k, jb in enumerate(pairs[gi]):
                        nc.tensor.matmul(
                            o_ps[h][64 * k : 64 * (k + 1), :],
                            vx[h][:, 128 * jb : 128 * jb + 64],
                            e[:, 512 * k : 512 * (k + 1)],
                            start=(gi == 0),
                            stop=(gi == npairs - 1),
                        )

                emit_S(0, 0)
                emit_S(1, 0)
                for gi in range(npairs):
                    emit_E(0, gi)
                    emit_E(1, gi)
                    if gi + 1 < npairs:
                        emit_S(0, gi + 1)
                        emit_S(1, gi + 1)
                    emit_O(0, gi)
                    # O(h1) deferred one period: by then its Schraudolph E is
                    # long done, so the in-order PE queue never stalls on VectorE
                    if gi > 0:
                        emit_O(1, gi - 1)
                    if mid_cb is not None and gi == npairs // 2:
                        mid_cb()
                emit_O(1, npairs - 1)
                return o_ps

            def emit_osb(qs, o_ps):
                qsl = slice(512 * qs, 512 * (qs + 1))
                for h in range(2):
                    # both osb evacuations on VectorE: on ScalarE this copy sits
                    # just ahead of the next unit's first exp and blocks it
                    nc.vector.tensor_copy(osb[h][:, qsl], o_ps[h][:, :])

            def emit_proj(qs):
                for h in range(2):
                    p = psS.tile([128, 1024], F32, name="pp", tag="s")
                    for lqb in range(4):
                        qb = 4 * qs + lqb
                        nc.tensor.matmul(
                            p[:, 256 * lqb : 256 * lqb + 132],
                            osb[h][:, 128 * qb : 128 * (qb + 1)],
                            wo[h][:, :],
                            start=True,
                            stop=True,
                        )
                    yb = ypool.tile([128, 4, 132], FP16, name="yb", tag="y")
                    src = p[:, 0:1024].rearrange("p (n s) -> p n s", s=256)[:, :, 0:132]
                    copy_h(h, yb[:, :, :], src)
                    dst = y_d[h, 4 * qs : 4 * qs + 4].rearrange("n p c -> p n c")
                    nc.sync.dma_start(dst, yb[:, :, :])

            with nc.named_scope("attn"):
                o_prev = None
                for qs in range(NQS):
                    if qs == 0:
                        emit_qkv(0)
                    if qs > 0:
                        emit_osb(qs - 1, o_prev)
                    # next q-super's qkv is emitted mid-unit so its PSUM->SBUF
                    # copies overlap this unit instead of stalling the boundary
                    cb = (lambda q=qs: emit_qkv(q + 1)) if qs + 1 < NQS else None
                    o_cur = emit_attn(qs, cb)
                    if qs > 0:
                        emit_proj(qs - 1)
                    o_prev = o_cur
                emit_osb(NQS - 1, o_prev)
                emit_proj(NQS - 1)

    nc.compile()
    return nc


def _to_bf16(x: np.ndarray) -> np.ndarray:
    import ml_dtypes

    return np.ascontiguousarray(x, dtype=np.float32).astype(ml_dtypes.bfloat16)


def make_in_maps(x: np.ndarray, W_qkv: np.ndarray, W_out: np.ndarray):
    x = np.asarray(x, dtype=np.float32)
    W_qkv = np.asarray(W_qkv, dtype=np.float32)
    W_out = np.asarray(W_out, dtype=np.float32)

    in_maps = []
    for c in range(NCORES):
        b = c // 2
        h0 = 2 * (c % 2)
        m = {"xt": _to_bf16(x[b].T), "zpad": _to_bf16(np.zeros((96, T), np.float32))}
        for i in range(2):
            h = h0 + i
            wqk = np.zeros((D, 64), np.float32)
            wqk[:, 0:32] = W_qkv[32 * h : 32 * h + 32, :].T
            wqk[:, 32:64] = W_qkv[128 + 32 * h : 128 + 32 * h + 32, :].T
            m[f"wqk{i}"] = _to_bf16(wqk)
            woi = np.zeros((128, 132), np.float32)
            woi[0:32, 0:128] = W_out[:, 32 * h : 32 * h + 32].T
            woi[64:96, 0:128] = W_out[:, 32 * h : 32 * h + 32].T
            woi[32, 128] = 1.0
            woi[96, 128] = 1.0
            m[f"wo{i}"] = woi.astype(np.float16)
        m["wv"] = _to_bf16(W_qkv[256 + 32 * h0 : 256 + 32 * h0 + 64, :].T)
        in_maps.append(m)
    return in_maps


_PROGRAM_CACHE = {}


def kernel(x: np.ndarray, W_qkv: np.ndarray, W_out: np.ndarray, _trace=False, _tmpdir=None) -> np.ndarray:
    if "nc" not in _PROGRAM_CACHE:
        _PROGRAM_CACHE["nc"] = build_program()
    nc = _PROGRAM_CACHE["nc"]

    in_maps = make_in_maps(x, W_qkv, W_out)
    res = bass_utils.run_bass_kernel_spmd(
        nc, in_maps, core_ids=list(range(NCORES)), trace=_trace, tmpdir=_tmpdir
    )
    out = np.zeros((B, T, D), np.float32)
    for c in range(NCORES):
        b = c // 2
        y = np.asarray(res.results[c]["y"], dtype=np.float32)  # [2, 32, 128, 132]
        for i in range(2):
            yi = y[i].reshape(T, 132)
            out[b] += yi[:, 0:128] / yi[:, 128:129]
    if _trace:
        kernel.last_result = res
    return out


# revision 44
# speedup vs baseline: 1.0290x; 1.0043x over previous
"""Trainium2 Bass kernel: causal multi-head self-attention (B=4, T=4096, D=128, H=4, dh=32).

Sharding: 8 cores = 4 batches x 2 head-pairs. Core c handles batch c//2, heads
{2*(c%2), 2*(c%2)+1}. Each core emits per-head unnormalized projections Y_h and
softmax denominators l_h; the host computes sum_h Y_h / l_h per batch.

All matmuls bf16/fp16, untiled full-array (keeps the PE HAM clock-gate warm at
2.4 GHz — row/col-tiled matmuls don't register as PE activity and the clock
throttles to 1.2 GHz). Per (head, q-super of 512 queries), j-blocks descending:
  S^T[j,q]   = ktz_jb(zero-padded K=128) @ qt  -> PSUM (diag blocks sliced to
               alive columns)
  E          : head 0 of the pair on ScalarE (exp); head 1 on VectorE via a
               custom DVE op relu(s*a+b) -> int16 whose bits read as bf16 give
               2^(s*log2e) (Schraudolph).
  causal mask: gpsimd affine_select zeroes j>q entries of E on diagonal blocks
               (emitted right after E, overlapped with later groups).
  O^T       += [V_j | ones | 0] @ E  (row 32 = softmax denominator l)
  proj       : Y[q,:128], l[q] = osb_block^T @ [W_out^T | ones-at-row-32]
"""

import math
import numpy as np

import concourse.bass as bass
import concourse.bacc as bacc
import concourse.mybir as mybir
import concourse.tile as tile
from concourse import bass_utils
import concourse.dve_ops as dve_ops
from concourse.dve_spec import Spec, Src0, C0, C1, relu, lower
from concourse.dve_uop import DveOpSpec

F32 = mybir.dt.float32
BF16 = mybir.dt.bfloat16
FP16 = mybir.dt.float16
I16 = mybir.dt.int16
Exp = mybir.ActivationFunctionType.Exp

B, T, D = 4, 4096, 128
H, DH = 4, 32
NCORES = 8
NQS = T // 512
SCALE = 1.0 / math.sqrt(DH)

# Schraudolph bf16-bit exp: bf16_bits(e^s) ~= round(s*log2e*128 + (127+sigma)*128)
SIGMA = -0.03
SCHR_A = (1.0 / math.log(2.0)) * SCALE * 128.0
SCHR_B = (127.0 + SIGMA) * 128.0


def _register_exp2():
    name = "EXP2_BITS_ANT"
    for op in dve_ops.OPS:
        if op.name == name:
            return op
    spec = Spec(body=relu(Src0 * C0 + C1))
    row = dve_ops._CUSTOM_DVE_ROW_BASE + len(dve_ops.OPS)
    assert row < 0x20
    shas = {}
    for ver in ("v3", "v4"):
        try:
            s = DveOpSpec(name=name, opcode=row, uops=lower(spec, ver=ver), rd1_en=False)
            shas[ver] = s.sha(ver)
        except Exception:
            pass
    dve_ops._SUB_OPCODE_FOR_NAME[name] = row
    op = dve_ops.DveOp(name, spec, subdim=False, uops_sha=shas)
    dve_ops.OPS.append(op)
    dve_ops.CUSTOM_DVE_SPECS[name] = spec
    return op


EXP2_OP = _register_exp2()


def build_program() -> bacc.Bacc:
    nc = bacc.Bacc("TRN2", target_bir_lowering=False, debug=False, num_devices=NCORES)

    xt_d = nc.dram_tensor("xt", [D, T], BF16, kind="ExternalInput").ap()
    wqk_d = [nc.dram_tensor(f"wqk{h}", [D, 64], BF16, kind="ExternalInput").ap() for h in range(2)]
    wv_d = nc.dram_tensor("wv", [D, 64], BF16, kind="ExternalInput").ap()
    wo_d = [nc.dram_tensor(f"wo{h}", [128, 132], FP16, kind="ExternalInput").ap() for h in range(2)]
    zpad_d = nc.dram_tensor("zpad", [96, T], BF16, kind="ExternalInput").ap()
    y_d = nc.dram_tensor("y", [2, NQS * 4, 128, 132], FP16, kind="ExternalOutput").ap()

    with tile.TileContext(nc) as tc:
        with (
            tc.tile_pool(name="const", bufs=1) as cpool,
            tc.tile_pool(name="epool", bufs=6) as epool,
            tc.tile_pool(name="ypool", bufs=4) as ypool,
            tc.tile_pool(name="psS", bufs=3, space="PSUM") as psS,
            tc.tile_pool(name="psO", bufs=2, space="PSUM") as psO,
        ):
            # ---- persistent SBUF ----
            xt = cpool.tile([D, T], BF16)
            wqk = [cpool.tile([D, 64], BF16, name=f"wqk{h}", tag=f"wqk{h}") for h in range(2)]
            wv = cpool.tile([D, 64], BF16)
            wo = [cpool.tile([128, 132], FP16, name=f"wo{h}", tag=f"wo{h}") for h in range(2)]
            qt = [cpool.tile([128, T], BF16, name=f"qt{h}", tag=f"qt{h}") for h in range(2)]
            ktz = [cpool.tile([128, T], BF16, name=f"ktz{h}", tag=f"ktz{h}") for h in range(2)]
            vx = [cpool.tile([128, 128 * 32], BF16, name=f"vx{h}", tag=f"vx{h}") for h in range(2)]
            osb = [cpool.tile([128, T], FP16, name=f"osb{h}", tag=f"osb{h}") for h in range(2)]

            # ---- init loads ----
            nc.sync.dma_start(xt[:, 0:512], xt_d[:, 0:512])
            for h in range(2):
                nc.sync.dma_start(wqk[h][:, :], wqk_d[h][:, :])
            for h in range(2):
                nc.scalar.dma_start(wo[h][:, :], wo_d[h][:, :])
            nc.scalar.dma_start(wv[:, :], wv_d[:, :])
            zq = [nc.sync, nc.gpsimd, nc.sync, nc.gpsimd]
            for h in range(2):
                # zero the padded contraction rows once; Q/K copies only write
                # rows 0:32. Issued from four different engine queues so the
                # descriptors dispatch in parallel instead of serializing the
                # startup on the Sync queue.
                zq[2 * h].dma_start(qt[h][32:128, :], zpad_d[:, :])
                zq[2 * h + 1].dma_start(ktz[h][32:128, :], zpad_d[:, :])
                # vx pattern: [V_j | ones | zeros] per 128-col block.
                # Unit 0's region (blocks 0-3) first so O(qs=0) isn't blocked;
                # the bulk zeroing goes to VectorE which is idle at startup.
                nc.gpsimd.memset(vx[h][:, 0:512], 0.0)
                for jb in range(4):
                    nc.gpsimd.memset(vx[h][:, 128 * jb + 32 : 128 * jb + 33], 1.0)
                nc.vector.memset(vx[h][:, 512:4096], 0.0)
                for jb in range(4, 32):
                    nc.gpsimd.memset(vx[h][:, 128 * jb + 32 : 128 * jb + 33], 1.0)

            def copy_h(h, out, in_):
                """PSUM->SBUF evacuations: head 0 on ScalarE, head 1 on VectorE."""
                if h == 0:
                    nc.scalar.copy(out, in_)
                else:
                    nc.vector.tensor_copy(out, in_)

            def emit_qkv(qs):
                qsl = slice(512 * qs, 512 * (qs + 1))
                for h in range(2):
                    p = psS.tile([128, 1024], F32, name="p", tag="s")
                    nc.tensor.matmul(p[0:64, 0:512], wqk[h][:, :], xt[:, qsl], start=True, stop=True)
                    # Q/K evacuations split by head: ScalarE is the period
                    # setter now (exp + aux), so h1's copies go to VectorE
                    copy_h(h, qt[h][0:32, qsl], p[0:32, 0:512])
                    copy_h(h, ktz[h][0:32, qsl], p[32:64, 0:512])
                pv = psS.tile([128, 1024], F32, name="pv", tag="s")
                for k in range(4):
                    jsl = slice(512 * qs + 128 * k, 512 * qs + 128 * (k + 1))
                    nc.tensor.matmul(pv[:, 64 * k : 64 * k + 64], xt[:, jsl], wv[:, :], start=True, stop=True)
                for h in range(2):
                    src = pv[:, 0:256].rearrange("p (n s) -> p n s", s=64)[:, :, 32 * h : 32 * h + 32]
                    dst = vx[h][:, 512 * qs * 1 : 512 * (qs + 1)].rearrange("p (n s) -> p n s", s=128)[:, :, 0:32]
                    nc.vector.tensor_copy(dst, src)
                if qs + 1 < NQS:
                    nsl = slice(512 * (qs + 1), 512 * (qs + 2))
                    nc.sync.dma_start(xt[:, nsl], xt_d[:, nsl])

            def emit_attn(qs, mid_cb=None):
                njb = 4 * (qs + 1)
                npairs = njb // 2
                o_ps = [psO.tile([128, 512], F32, name=f"o{h}", tag="o") for h in range(2)]
                s_tiles = {}
                e_tiles = {}
                blocks = list(range(njb - 1, -1, -1))  # descending: diag first
                pairs = [blocks[2 * i : 2 * i + 2] for i in range(npairs)]

                def emit_S(h, gi):
                    s = psS.tile([128, 1024], F32, name="s", tag="s")
                    s_tiles[(h, gi)] = s
                    for k, jb in enumerate(pairs[gi]):
                        g = jb - 4 * qs
                        lo = 128 * g if g > 0 else 0
                        nc.tensor.matmul(
                            s[:, 512 * k + lo : 512 * (k + 1)],
                            ktz[h][:, 128 * jb : 128 * (jb + 1)],
                            qt[h][:, 512 * qs + lo : 512 * (qs + 1)],
                            start=True,
                            stop=True,
                        )

                def emit_E(h, gi):
                    s = s_tiles.pop((h, gi))
                    e = epool.tile([128, 1024], BF16, name="e", tag="e")
                    e_tiles[(h, gi)] = e
                    # contiguous dead-column skip: pair 0 (g=3,2) cols [0:384)
                    # and pair 1 (g=1,0) cols [0:128) are all overwritten to 0
                    # by affine_select, so E need not be computed there
                    lo = 384 if gi == 0 else (128 if gi == 1 else 0)
                    if h == 0:
                        nc.scalar.activation(e[:, lo:1024], s[:, lo:1024], Exp, scale=SCALE)
                    elif gi == 0:
                        # first pair = diag blocks g=3,2: columns below the
                        # causal frontier are overwritten to 0 by affine_select
                        # anyway, so the Schraudolph op skips them (384 of 1024)
                        for sl in (slice(384, 512), slice(768, 1024)):
                            nc.vector._custom_dve(
                                EXP2_OP, out=e[:, sl].bitcast(I16), in0=s[:, sl], s0=SCHR_A, s1=SCHR_B
                            )
                    else:
                        sl = slice(lo, 1024)
                        nc.vector._custom_dve(
                            EXP2_OP, out=e[:, sl].bitcast(I16), in0=s[:, sl], s0=SCHR_A, s1=SCHR_B
                        )
                    # causal mask: zero E where 128g + p > l on diagonal blocks
                    for k, jb in enumerate(pairs[gi]):
                        g = jb - 4 * qs
                        if g >= 0:
                            w = 128 * (g + 1)
                            nc.gpsimd.affine_select(
                                e[:, 512 * k : 512 * k + w],
                                e[:, 512 * k : 512 * k + w],
                                pattern=[[1, w]],
                                compare_op=mybir.AluOpType.is_ge,
                                fill=0.0,
                                base=-128 * g,
                                channel_multiplier=-1,
                            )

                def emit_O(h, gi):
                    e = e_tiles.pop((h, gi))
                    # 2x col-tiled: block k accumulates into partition strip
                    # [64k, 64k+64); strips are summed for free in the output
                    # projection via W_out replicated along partitions.
                    for k, jb in enumerate(pairs[gi]):
                        nc.tensor.matmul(
                            o_ps[h][64 * k : 64 * (k + 1), :],
                            vx[h][:, 128 * jb : 128 * jb + 64],
                            e[:, 512 * k : 512 * (k + 1)],
                            start=(gi == 0),
                            stop=(gi == npairs - 1),
                        )

                emit_S(0, 0)
                emit_S(1, 0)
                for gi in range(npairs):
                    emit_E(0, gi)
                    emit_E(1, gi)
                    if gi + 1 < npairs:
                        emit_S(0, gi + 1)
                        emit_S(1, gi + 1)
                    emit_O(0, gi)
                    # O(h1) deferred one period: by then its Schraudolph E is
                    # long done, so the in-order PE queue never stalls on VectorE
                    if gi > 0:
                        emit_O(1, gi - 1)
                    if mid_cb is not None and gi == npairs // 2:
                        mid_cb()
                emit_O(1, npairs - 1)
                return o_ps

            def emit_osb(qs, o_ps):
                qsl = slice(512 * qs, 512 * (qs + 1))
                for h in range(2):
                    # both osb evacuations on VectorE: on ScalarE this copy sits
                    # just ahead of the next unit's first exp and blocks it
                    nc.vector.tensor_copy(osb[h][:, qsl], o_ps[h][:, :])

            def emit_proj(qs):
                for h in range(2):
                    p = psS.tile([128, 1024], F32, name="pp", tag="s")
                    for lqb in range(4):
                        qb = 4 * qs + lqb
                        nc.tensor.matmul(
                            p[:, 256 * lqb : 256 * lqb + 132],
                            osb[h][:, 128 * qb : 128 * (qb + 1)],
                            wo[h][:, :],
                            start=True,
                            stop=True,
                        )
                    yb = ypool.tile([128, 4, 132], FP16, name="yb", tag="y")
                    src = p[:, 0:1024].rearrange("p (n s) -> p n s", s=256)[:, :, 0:132]
                    copy_h(h, yb[:, :, :], src)
                    dst = y_d[h, 4 * qs : 4 * qs + 4].rearrange("n p c -> p n c")
                    nc.sync.dma_start(dst, yb[:, :, :])

            with nc.named_scope("attn"):
                o_prev = None
                for qs in range(NQS):
                    if qs == 0:
                        emit_qkv(0)
                    if qs > 0:
                        emit_osb(qs - 1, o_prev)
                    # next q-super's qkv is emitted mid-unit so its PSUM->SBUF
                    # copies overlap this unit instead of stalling the boundary
                    cb = (lambda q=qs: emit_qkv(q + 1)) if qs + 1 < NQS else None
                    o_cur = emit_attn(qs, cb)
                    if qs > 0:
                        emit_proj(qs - 1)
                    o_prev = o_cur
                emit_osb(NQS - 1, o_prev)
                emit_proj(NQS - 1)

    nc.compile()
    return nc


def _to_bf16(x: np.ndarray) -> np.ndarray:
    import ml_dtypes

    return np.ascontiguousarray(x, dtype=np.float32).astype(ml_dtypes.bfloat16)


def make_in_maps(x: np.ndarray, W_qkv: np.ndarray, W_out: np.ndarray):
    x = np.asarray(x, dtype=np.float32)
    W_qkv = np.asarray(W_qkv, dtype=np.float32)
    W_out = np.asarray(W_out, dtype=np.float32)

    in_maps = []
    for c in range(NCORES):
        b = c // 2
        h0 = 2 * (c % 2)
        m = {"xt": _to_bf16(x[b].T), "zpad": _to_bf16(np.zeros((96, T), np.float32))}
        for i in range(2):
            h = h0 + i
            wqk = np.zeros((D, 64), np.float32)
            wqk[:, 0:32] = W_qkv[32 * h : 32 * h + 32, :].T
            wqk[:, 32:64] = W_qkv[128 + 32 * h : 128 + 32 * h + 32, :].T
            m[f"wqk{i}"] = _to_bf16(wqk)
            woi = np.zeros((128, 132), np.float32)
            woi[0:32, 0:128] = W_out[:, 32 * h : 32 * h + 32].T
            woi[64:96, 0:128] = W_out[:, 32 * h : 32 * h + 32].T
            woi[32, 128] = 1.0
            woi[96, 128] = 1.0
            m[f"wo{i}"] = woi.astype(np.float16)
        m["wv"] = _to_bf16(W_qkv[256 + 32 * h0 : 256 + 32 * h0 + 64, :].T)
        in_maps.append(m)
    return in_maps


_PROGRAM_CACHE = {}


def kernel(x: np.ndarray, W_qkv: np.ndarray, W_out: np.ndarray, _trace=False, _tmpdir=None) -> np.ndarray:
    if "nc" not in _PROGRAM_CACHE:
        _PROGRAM_CACHE["nc"] = build_program()
    nc = _PROGRAM_CACHE["nc"]

    in_maps = make_in_maps(x, W_qkv, W_out)
    res = bass_utils.run_bass_kernel_spmd(
        nc, in_maps, core_ids=list(range(NCORES)), trace=_trace, tmpdir=_tmpdir
    )
    out = np.zeros((B, T, D), np.float32)
    for c in range(NCORES):
        b = c // 2
        y = np.asarray(res.results[c]["y"], dtype=np.float32)  # [2, 32, 128, 132]
        for i in range(2):
            yi = y[i].reshape(T, 132)
            out[b] += yi[:, 0:128] / yi[:, 128:129]
    if _trace:
        kernel.last_result = res
    return out


# revision 45
# speedup vs baseline: 1.0440x; 1.0146x over previous
"""Trainium2 Bass kernel: causal multi-head self-attention (B=4, T=4096, D=128, H=4, dh=32).

Sharding: 8 cores = 4 batches x 2 head-pairs. Core c handles batch c//2, heads
{2*(c%2), 2*(c%2)+1}. Each core emits per-head unnormalized projections Y_h and
softmax denominators l_h; the host computes sum_h Y_h / l_h per batch.

All matmuls bf16/fp16, untiled full-array (keeps the PE HAM clock-gate warm at
2.4 GHz — row/col-tiled matmuls don't register as PE activity and the clock
throttles to 1.2 GHz). Per (head, q-super of 512 queries), j-blocks descending:
  S^T[j,q]   = ktz_jb(zero-padded K=128) @ qt  -> PSUM (diag blocks sliced to
               alive columns)
  E          : head 0 of the pair on ScalarE (exp); head 1 on VectorE via a
               custom DVE op relu(s*a+b) -> int16 whose bits read as bf16 give
               2^(s*log2e) (Schraudolph).
  causal mask: gpsimd affine_select zeroes j>q entries of E on diagonal blocks
               (emitted right after E, overlapped with later groups).
  O^T       += [V_j | ones | 0] @ E  (row 32 = softmax denominator l)
  proj       : Y[q,:128], l[q] = osb_block^T @ [W_out^T | ones-at-row-32]
"""

import math
import numpy as np

import concourse.bass as bass
import concourse.bacc as bacc
import concourse.mybir as mybir
import concourse.tile as tile
from concourse import bass_utils
import concourse.dve_ops as dve_ops
from concourse.dve_spec import Spec, Src0, C0, C1, relu, lower
from concourse.dve_uop import DveOpSpec

F32 = mybir.dt.float32
BF16 = mybir.dt.bfloat16
FP16 = mybir.dt.float16
I16 = mybir.dt.int16
Exp = mybir.ActivationFunctionType.Exp

B, T, D = 4, 4096, 128
H, DH = 4, 32
NCORES = 8
NQS = T // 512
SCALE = 1.0 / math.sqrt(DH)

# Schraudolph bf16-bit exp: bf16_bits(e^s) ~= round(s*log2e*128 + (127+sigma)*128)
SIGMA = -0.03
SCHR_A = (1.0 / math.log(2.0)) * SCALE * 128.0
SCHR_B = (127.0 + SIGMA) * 128.0


def _register_exp2():
    name = "EXP2_BITS_ANT"
    for op in dve_ops.OPS:
        if op.name == name:
            return op
    spec = Spec(body=relu(Src0 * C0 + C1))
    row = dve_ops._CUSTOM_DVE_ROW_BASE + len(dve_ops.OPS)
    assert row < 0x20
    shas = {}
    for ver in ("v3", "v4"):
        try:
            s = DveOpSpec(name=name, opcode=row, uops=lower(spec, ver=ver), rd1_en=False)
            shas[ver] = s.sha(ver)
        except Exception:
            pass
    dve_ops._SUB_OPCODE_FOR_NAME[name] = row
    op = dve_ops.DveOp(name, spec, subdim=False, uops_sha=shas)
    dve_ops.OPS.append(op)
    dve_ops.CUSTOM_DVE_SPECS[name] = spec
    return op


EXP2_OP = _register_exp2()


def build_program() -> bacc.Bacc:
    nc = bacc.Bacc("TRN2", target_bir_lowering=False, debug=False, num_devices=NCORES)

    xt_d = nc.dram_tensor("xt", [D, T], BF16, kind="ExternalInput").ap()
    wqk_d = [nc.dram_tensor(f"wqk{h}", [D, 64], BF16, kind="ExternalInput").ap() for h in range(2)]
    wv_d = nc.dram_tensor("wv", [D, 64], BF16, kind="ExternalInput").ap()
    wo_d = [nc.dram_tensor(f"wo{h}", [128, 132], FP16, kind="ExternalInput").ap() for h in range(2)]
    zpad_d = nc.dram_tensor("zpad", [96, T], BF16, kind="ExternalInput").ap()
    y_d = nc.dram_tensor("y", [2, NQS * 4, 128, 132], FP16, kind="ExternalOutput").ap()

    with tile.TileContext(nc) as tc:
        with (
            tc.tile_pool(name="const", bufs=1) as cpool,
            tc.tile_pool(name="epool", bufs=6) as epool,
            tc.tile_pool(name="ypool", bufs=4) as ypool,
            tc.tile_pool(name="psS", bufs=3, space="PSUM") as psS,
            tc.tile_pool(name="psO", bufs=2, space="PSUM") as psO,
        ):
            # ---- persistent SBUF ----
            xt = cpool.tile([D, T], BF16)
            wqk = [cpool.tile([D, 64], BF16, name=f"wqk{h}", tag=f"wqk{h}") for h in range(2)]
            wv = cpool.tile([D, 64], BF16)
            wo = [cpool.tile([128, 132], FP16, name=f"wo{h}", tag=f"wo{h}") for h in range(2)]
            qt = [cpool.tile([128, T], BF16, name=f"qt{h}", tag=f"qt{h}") for h in range(2)]
            ktz = [cpool.tile([128, T], BF16, name=f"ktz{h}", tag=f"ktz{h}") for h in range(2)]
            vx = [cpool.tile([128, 128 * 32], BF16, name=f"vx{h}", tag=f"vx{h}") for h in range(2)]
            osb = [cpool.tile([128, T], FP16, name=f"osb{h}", tag=f"osb{h}") for h in range(2)]

            # ---- init loads ----
            nc.sync.dma_start(xt[:, 0:512], xt_d[:, 0:512])
            for h in range(2):
                nc.sync.dma_start(wqk[h][:, :], wqk_d[h][:, :])
            for h in range(2):
                nc.scalar.dma_start(wo[h][:, :], wo_d[h][:, :])
            nc.scalar.dma_start(wv[:, :], wv_d[:, :])
            zq = [nc.sync, nc.gpsimd, nc.sync, nc.gpsimd]
            for h in range(2):
                # zero the padded contraction rows once; Q/K copies only write
                # rows 0:32. Issued from four different engine queues so the
                # descriptors dispatch in parallel instead of serializing the
                # startup on the Sync queue.
                zq[2 * h].dma_start(qt[h][32:128, :], zpad_d[:, :])
                zq[2 * h + 1].dma_start(ktz[h][32:128, :], zpad_d[:, :])
                # vx pattern: [V_j | ones | zeros] per 128-col block.
                # Unit 0's region (blocks 0-3) first so O(qs=0) isn't blocked;
                # the bulk zeroing goes to VectorE which is idle at startup.
                nc.gpsimd.memset(vx[h][:, 0:512], 0.0)
                for jb in range(4):
                    nc.gpsimd.memset(vx[h][:, 128 * jb + 32 : 128 * jb + 33], 1.0)
                nc.vector.memset(vx[h][:, 512:4096], 0.0)
                for jb in range(4, 32):
                    nc.gpsimd.memset(vx[h][:, 128 * jb + 32 : 128 * jb + 33], 1.0)

            def copy_h(h, out, in_):
                """PSUM->SBUF evacuations: head 0 on ScalarE, head 1 on VectorE."""
                if h == 0:
                    nc.scalar.copy(out, in_)
                else:
                    nc.vector.tensor_copy(out, in_)

            def emit_qkv(qs):
                qsl = slice(512 * qs, 512 * (qs + 1))
                for h in range(2):
                    p = psS.tile([128, 1024], F32, name="p", tag="s")
                    nc.tensor.matmul(p[0:64, 0:512], wqk[h][:, :], xt[:, qsl], start=True, stop=True)
                    # Q/K evacuations split by head: ScalarE is the period
                    # setter now (exp + aux), so h1's copies go to VectorE
                    copy_h(h, qt[h][0:32, qsl], p[0:32, 0:512])
                    copy_h(h, ktz[h][0:32, qsl], p[32:64, 0:512])
                pv = psS.tile([128, 1024], F32, name="pv", tag="s")
                for k in range(4):
                    jsl = slice(512 * qs + 128 * k, 512 * qs + 128 * (k + 1))
                    nc.tensor.matmul(pv[:, 64 * k : 64 * k + 64], xt[:, jsl], wv[:, :], start=True, stop=True)
                for h in range(2):
                    src = pv[:, 0:256].rearrange("p (n s) -> p n s", s=64)[:, :, 32 * h : 32 * h + 32]
                    dst = vx[h][:, 512 * qs * 1 : 512 * (qs + 1)].rearrange("p (n s) -> p n s", s=128)[:, :, 0:32]
                    nc.vector.tensor_copy(dst, src)
                if qs + 1 < NQS:
                    nsl = slice(512 * (qs + 1), 512 * (qs + 2))
                    nc.sync.dma_start(xt[:, nsl], xt_d[:, nsl])

            def emit_attn(qs, mid_cb=None):
                njb = 4 * (qs + 1)
                npairs = njb // 2
                o_ps = [psO.tile([128, 512], F32, name=f"o{h}", tag="o") for h in range(2)]
                s_tiles = {}
                e_tiles = {}
                blocks = list(range(njb - 1, -1, -1))  # descending: diag first
                pairs = [blocks[2 * i : 2 * i + 2] for i in range(npairs)]

                def emit_S(h, gi):
                    s = psS.tile([128, 1024], F32, name="s", tag="s")
                    s_tiles[(h, gi)] = s
                    for k, jb in enumerate(pairs[gi]):
                        g = jb - 4 * qs
                        lo = 128 * g if g > 0 else 0
                        nc.tensor.matmul(
                            s[:, 512 * k + lo : 512 * (k + 1)],
                            ktz[h][:, 128 * jb : 128 * (jb + 1)],
                            qt[h][:, 512 * qs + lo : 512 * (qs + 1)],
                            start=True,
                            stop=True,
                        )

                def emit_E(h, gi):
                    s = s_tiles.pop((h, gi))
                    e = epool.tile([128, 1024], BF16, name="e", tag="e")
                    e_tiles[(h, gi)] = e
                    # contiguous dead-column skip: pair 0 (g=3,2) cols [0:384)
                    # and pair 1 (g=1,0) cols [0:128) are all overwritten to 0
                    # by affine_select, so E need not be computed there
                    lo = 384 if gi == 0 else (128 if gi == 1 else 0)
                    if h == 0:
                        nc.scalar.activation(e[:, lo:1024], s[:, lo:1024], Exp, scale=SCALE)
                    elif gi == 0:
                        # first pair = diag blocks g=3,2: columns below the
                        # causal frontier are overwritten to 0 by affine_select
                        # anyway, so the Schraudolph op skips them (384 of 1024)
                        for sl in (slice(384, 512), slice(768, 1024)):
                            nc.vector._custom_dve(
                                EXP2_OP, out=e[:, sl].bitcast(I16), in0=s[:, sl], s0=SCHR_A, s1=SCHR_B
                            )
                    else:
                        sl = slice(lo, 1024)
                        nc.vector._custom_dve(
                            EXP2_OP, out=e[:, sl].bitcast(I16), in0=s[:, sl], s0=SCHR_A, s1=SCHR_B
                        )
                    # causal mask: zero E where 128g + p > l on diagonal blocks
                    for k, jb in enumerate(pairs[gi]):
                        g = jb - 4 * qs
                        if g >= 0:
                            w = 128 * (g + 1)
                            nc.gpsimd.affine_select(
                                e[:, 512 * k : 512 * k + w],
                                e[:, 512 * k : 512 * k + w],
                                pattern=[[1, w]],
                                compare_op=mybir.AluOpType.is_ge,
                                fill=0.0,
                                base=-128 * g,
                                channel_multiplier=-1,
                            )

                def emit_O(h, gi):
                    e = e_tiles.pop((h, gi))
                    # 2x col-tiled: block k accumulates into partition strip
                    # [64k, 64k+64); strips are summed for free in the output
                    # projection via W_out replicated along partitions.
                    for k, jb in enumerate(pairs[gi]):
                        nc.tensor.matmul(
                            o_ps[h][64 * k : 64 * (k + 1), :],
                            vx[h][:, 128 * jb : 128 * jb + 64],
                            e[:, 512 * k : 512 * (k + 1)],
                            start=(gi == 0),
                            stop=(gi == npairs - 1),
                        )

                emit_S(0, 0)
                emit_S(1, 0)
                for gi in range(npairs):
                    emit_E(0, gi)
                    emit_E(1, gi)
                    if gi + 1 < npairs:
                        emit_S(0, gi + 1)
                        emit_S(1, gi + 1)
                    emit_O(0, gi)
                    # O(h1) deferred two periods: by then its Schraudolph E is
                    # long done, so the in-order PE queue never stalls on
                    # VectorE (epool's 6 slots = 3 per head exactly cover this)
                    if gi > 1:
                        emit_O(1, gi - 2)
                    if mid_cb is not None and gi == npairs // 2:
                        mid_cb()
                emit_O(1, npairs - 2)
                emit_O(1, npairs - 1)
                return o_ps

            def emit_osb(qs, o_ps):
                qsl = slice(512 * qs, 512 * (qs + 1))
                for h in range(2):
                    # both osb evacuations on VectorE: on ScalarE this copy sits
                    # just ahead of the next unit's first exp and blocks it
                    nc.vector.tensor_copy(osb[h][:, qsl], o_ps[h][:, :])

            def emit_proj(qs):
                for h in range(2):
                    p = psS.tile([128, 1024], F32, name="pp", tag="s")
                    for lqb in range(4):
                        qb = 4 * qs + lqb
                        nc.tensor.matmul(
                            p[:, 256 * lqb : 256 * lqb + 132],
                            osb[h][:, 128 * qb : 128 * (qb + 1)],
                            wo[h][:, :],
                            start=True,
                            stop=True,
                        )
                    yb = ypool.tile([128, 4, 132], FP16, name="yb", tag="y")
                    src = p[:, 0:1024].rearrange("p (n s) -> p n s", s=256)[:, :, 0:132]
                    copy_h(h, yb[:, :, :], src)
                    dst = y_d[h, 4 * qs : 4 * qs + 4].rearrange("n p c -> p n c")
                    nc.sync.dma_start(dst, yb[:, :, :])

            with nc.named_scope("attn"):
                o_prev = None
                for qs in range(NQS):
                    if qs == 0:
                        emit_qkv(0)
                    if qs > 0:
                        emit_osb(qs - 1, o_prev)
                    # next q-super's qkv is emitted mid-unit so its PSUM->SBUF
                    # copies overlap this unit instead of stalling the boundary
                    cb = (lambda q=qs: emit_qkv(q + 1)) if qs + 1 < NQS else None
                    o_cur = emit_attn(qs, cb)
                    if qs > 0:
                        emit_proj(qs - 1)
                    o_prev = o_cur
                emit_osb(NQS - 1, o_prev)
                emit_proj(NQS - 1)

    nc.compile()
    return nc


def _to_bf16(x: np.ndarray) -> np.ndarray:
    import ml_dtypes

    return np.ascontiguousarray(x, dtype=np.float32).astype(ml_dtypes.bfloat16)


def make_in_maps(x: np.ndarray, W_qkv: np.ndarray, W_out: np.ndarray):
    x = np.asarray(x, dtype=np.float32)
    W_qkv = np.asarray(W_qkv, dtype=np.float32)
    W_out = np.asarray(W_out, dtype=np.float32)

    in_maps = []
    for c in range(NCORES):
        b = c // 2
        h0 = 2 * (c % 2)
        m = {"xt": _to_bf16(x[b].T), "zpad": _to_bf16(np.zeros((96, T), np.float32))}
        for i in range(2):
            h = h0 + i
            wqk = np.zeros((D, 64), np.float32)
            wqk[:, 0:32] = W_qkv[32 * h : 32 * h + 32, :].T
            wqk[:, 32:64] = W_qkv[128 + 32 * h : 128 + 32 * h + 32, :].T
            m[f"wqk{i}"] = _to_bf16(wqk)
            woi = np.zeros((128, 132), np.float32)
            woi[0:32, 0:128] = W_out[:, 32 * h : 32 * h + 32].T
            woi[64:96, 0:128] = W_out[:, 32 * h : 32 * h + 32].T
            woi[32, 128] = 1.0
            woi[96, 128] = 1.0
            m[f"wo{i}"] = woi.astype(np.float16)
        m["wv"] = _to_bf16(W_qkv[256 + 32 * h0 : 256 + 32 * h0 + 64, :].T)
        in_maps.append(m)
    return in_maps


_PROGRAM_CACHE = {}


def kernel(x: np.ndarray, W_qkv: np.ndarray, W_out: np.ndarray, _trace=False, _tmpdir=None) -> np.ndarray:
    if "nc" not in _PROGRAM_CACHE:
        _PROGRAM_CACHE["nc"] = build_program()
    nc = _PROGRAM_CACHE["nc"]

    in_maps = make_in_maps(x, W_qkv, W_out)
    res = bass_utils.run_bass_kernel_spmd(
        nc, in_maps, core_ids=list(range(NCORES)), trace=_trace, tmpdir=_tmpdir
    )
    out = np.zeros((B, T, D), np.float32)
    for c in range(NCORES):
        b = c // 2
        y = np.asarray(res.results[c]["y"], dtype=np.float32)  # [2, 32, 128, 132]
        for i in range(2):
            yi = y[i].reshape(T, 132)
            out[b] += yi[:, 0:128] / yi[:, 128:129]
    if _trace:
        kernel.last_result = res
    return out
